# revision 26
# baseline (speedup 1.0000x reference)
"""Trainium2 Bass kernel for nn_Criterion_8761733284571.

Pairwise Wasserstein-attention similarity (1-step Sinkhorn) + multisimilarity
loss over a 64-sample batch.

v3 design ("no-bounce"), consolidated:
  - Symmetry: converged transport satisfies sim(i,j) = sim(j,i); only pairs
    with circular distance d = (j-i) mod 64 in [1,32] are computed (validated
    rel-err 3.5e-3 vs reference).  8 anchors/core x 32 d's = 256 pairs/core.
  - K stays in Gram layout [98=(anchor-half, s), (j-window, m)]: row-sums are
    contiguous 49-grouped DVE reduces; column reductions (den_c, SB) are PE
    ones-matmuls over the partition dim.  No big pair-major transpose.
  - Sinkhorn scale invariance: r = u/den0 (the u-normalization cancels in
    sum_m SB[m]*v[m]/den_c[m]); no partition-broadcast of per-pair sums.
  - Host precomputes normalized xn (bf16, only the 40 used j-columns),
    attention marginals u (Gram window layout) and v (pair-major, divided by
    vsum+1e-5), and runs the 64x64 multisimilarity reduction on gathered sims.
  - W and P2=W*sim live interleaved in one tensor so den_c|SB come from one
    7-chunk ones-matmul pass and one fused 196B-descriptor scatter per half.
"""

import os as _os

import numpy as np
from contextlib import ExitStack

import concourse.bass as bass
import concourse.bacc as bacc
import concourse.mybir as mybir
import concourse.tile as tile

F32 = mybir.dt.float32
BF16 = mybir.dt.bfloat16
AF = mybir.ActivationFunctionType
ALU = mybir.AluOpType
AX = mybir.AxisListType

B = 64          # batch
C = 128         # channels
S = 49          # spatial (7*7)
NCORES = 8
IPC = B // NCORES       # anchors per core = 8
NT = IPC // 2           # anchor-pair tiles = 4
ND = 32                 # circular distances per anchor
NW = ND + 1             # j-window width per tile = 33
WCOL = NW * S           # 1617
NJ = 2 * NT + ND        # j-columns actually used per core = 40
COLS = NJ * S           # 1960
TBLK = 2                # pair-major blocks of 128 pairs
PP = 2 * S              # 98 partitions in Gram layout

POS_W = 2.0
NEG_W = 40.0
MARGIN = 0.1
THRESH = 0.5


def _bc(ap, pos, count):
    """Insert a stride-0 (broadcast) dim of size `count` at position `pos`."""
    new = ap.ap[:pos] + [[0, count]] + ap.ap[pos:]
    return bass.AP(tensor=ap.tensor, offset=ap.offset, ap=new)


def _t11(ap):
    """Append a trailing [1, 1] dim (keeps the DVE 2x datapath with a
    broadcast operand; see the v2 kernel's measured bc-mul timings)."""
    return bass.AP(tensor=ap.tensor, offset=ap.offset, ap=ap.ap + [[1, 1]])


def _body(ctx, tc, io):
    nc = tc.nc

    psm = ctx.enter_context(tc.tile_pool(name="psm", bufs=1))
    ppg = ctx.enter_context(tc.tile_pool(name="ppg", bufs=2, space="PSUM"))
    ppr = ctx.enter_context(tc.tile_pool(name="ppr", bufs=2, space="PSUM"))

    # ---- constants ----
    cm20 = psm.tile([128, 1], F32)
    nc.vector.memset(cm20[:], -20.0)
    ones2 = psm.tile([PP, 2], BF16)

    # ---- load inputs (xnb chunked so tile 0's window lands first) ----
    xnb = psm.tile([C, COLS], BF16, tag="XNB")
    XCH = [(0, 537), (537, 1073), (1073, 1666), (1666, COLS)]
    for i, (x0, x1) in enumerate(XCH):
        eng = nc.sync if i % 2 == 0 else nc.scalar
        eng.dma_start(xnb[:, x0:x1], io["xnb"][:, x0:x1])
    nc.sync.dma_start(ones2[:], io["ones2"][:])
    uG = psm.tile([PP, NT, NW], F32)
    nc.scalar.dma_start(uG[:], io["ug"][:])
    vP = psm.tile([128, TBLK, S], F32)
    nc.sync.dma_start(vP[:], io["vp"][:])

    # ---- big per-tile tensors (Gram layout) ----
    simS = psm.tile([PP, NT, WCOL], BF16, tag="SIMS")
    KG = psm.tile([PP, NT, WCOL], BF16, tag="KG")
    WP = psm.tile([PP, NT, NW, 2, S], BF16, tag="WP")  # per jw: [W | W*sim]
    den0 = psm.tile([PP, NT, NW], F32)
    di0 = psm.tile([PP, NT, NW], F32)
    r0 = psm.tile([PP, NT, NW], BF16)
    dsbs = psm.tile([2, NT, 2 * WCOL], F32)            # den_c | SB interleaved
    dsP = psm.tile([128, TBLK, 2, S], F32)             # pair-major den_c, SB
    rdc = psm.tile([128, TBLK, S], F32)
    ct = psm.tile([128, TBLK, S], F32)
    dotv = psm.tile([128, TBLK], F32)

    # Gram PSUM pieces (2 banks each) and ones-reduce pieces (4 banks each)
    GP = [(0, 1024), (1024, WCOL)]
    # piece order: DVE-drained pieces first so DVE's copies are ready
    # before it must start the next tile's den0 (avoids head-of-line stalls)
    RP = [(1024, 2048, "v"), (3072, 2 * WCOL, "v"),
          (0, 1024, "a"), (2048, 3072, "a")]

    for t in range(NT):
        a0 = 2 * t * S
        w0 = (2 * t + 1) * S
        # Gram: [98 anchor-cols, 33-j window]; per PSUM piece: matmuls at
        # bank-aligned 512-subslices, then one copy (sim) + one exp (K)
        for (g0, g1) in GP:
            pg = ppg.tile([PP, 1024], F32, tag="pg")
            for f0 in range(g0, g1, 512):
                f1 = min(f0 + 512, g1)
                nc.tensor.matmul(pg[:, f0 - g0:f1 - g0],
                                 lhsT=xnb[:, a0:a0 + PP],
                                 rhs=xnb[:, w0 + f0:w0 + f1],
                                 start=True, stop=True)
            nc.scalar.copy(simS[:, t, g0:g1], pg[:, 0:g1 - g0])
            nc.scalar.activation(KG[:, t, g0:g1], pg[:, 0:g1 - g0], AF.Exp,
                                 bias=cm20[0:PP], scale=20.0)
        # den0[s, jw] = sum_m K ; r0 = u_raw / den0 (usum cancels downstream)
        nc.vector.tensor_reduce(
            den0[:, t], KG[:, t].rearrange("p (j m) -> p j m", m=S),
            axis=AX.X, op=ALU.add)
        nc.vector.reciprocal(di0[:, t], den0[:, t])
        nc.vector.tensor_mul(r0[:, t], uG[:, t], di0[:, t])
        # W = K * r0 (r0 broadcast along m); P2 = W * sim (gpsimd, off path)
        nc.vector.tensor_mul(
            WP[:, t, :, 0],
            KG[:, t].rearrange("p (j m) -> p j m", m=S),
            _bc(r0[:, t], 2, S))
        if t == 2:
            _finish(nc, 0, dsP, vP, rdc, ct, dotv, io)
        p2eng = nc.gpsimd if t < NT - 1 else nc.vector
        p2eng.tensor_mul(WP[:, t, :, 1], WP[:, t, :, 0],
                         simS[:, t].rearrange("p (j m) -> p j m", m=S))
        # den_c | SB: PE ones-reduce over partitions; per 4-bank PSUM piece:
        # matmuls at bank-aligned 512-subslices, then one big copy
        wp_flat = WP[:, t].rearrange("p j c m -> p (j c m)")
        for (g0, g1, weng) in RP:
            pd = ppr.tile([2, 1024], F32, tag="pr")
            for f0 in range(g0, g1, 512):
                f1 = min(f0 + 512, g1)
                nc.tensor.matmul(pd[:, f0 - g0:f1 - g0], lhsT=ones2[:],
                                 rhs=wp_flat[:, f0:f1],
                                 start=True, stop=True)
            if weng == "a":
                nc.scalar.copy(dsbs[:, t, g0:g1], pd[:, 0:g1 - g0])
            else:
                nc.vector.tensor_copy(dsbs[:, t, g0:g1], pd[:, 0:g1 - g0])
        # scatter den_c/SB to pair-major [128, 2, 49]; one DMA per half
        b = t // 2
        qb = (t % 2) * 64
        for h in range(2):
            src = dsbs[h:h + 1, t].rearrange("p (j w) -> p j w", w=2 * S)
            src = bass.AP(tensor=src.tensor,
                          offset=src.offset + h * 2 * S,
                          ap=[src.ap[0], [src.ap[1][0], ND], src.ap[2]])
            nc.sync.dma_start(dsP[qb + h * ND:qb + (h + 1) * ND, b], src)

    _finish(nc, 1, dsP, vP, rdc, ct, dotv, io)


def _finish(nc, b, dsP, vP, rdc, ct, dotv, io):
    # pair-major finish: c = vP / den_c ; dot = sum_m SB * c
    nc.vector.reciprocal(rdc[:, b], dsP[:, b, 0])
    nc.vector.tensor_mul(ct[:, b], vP[:, b], rdc[:, b])
    nc.vector.tensor_mul(ct[:, b], ct[:, b], dsP[:, b, 1])
    nc.vector.tensor_reduce(dotv[:, b:b + 1], ct[:, b],
                            axis=AX.X, op=ALU.add)
    nc.scalar.dma_start(io["dot"][:, b:b + 1], dotv[:, b:b + 1])


def build_nc():
    nc = bacc.Bacc("TRN2", target_bir_lowering=False, debug=False)
    io = {}
    io["xnb"] = nc.declare_dram_parameter("xnb", [C, COLS], BF16, isOutput=False)
    io["ones2"] = nc.declare_dram_parameter("ones2", [PP, 2], BF16, isOutput=False)
    io["ug"] = nc.declare_dram_parameter("ug", [PP, NT, NW], F32, isOutput=False)
    io["vp"] = nc.declare_dram_parameter("vp", [128, TBLK, S], F32, isOutput=False)
    io["dot"] = nc.declare_dram_parameter("dot", [128, TBLK], F32, isOutput=True)
    with tile.TileContext(nc) as tc, ExitStack() as ctx:
        _body(ctx, tc, io)
    nc.compile()
    return nc


_NC_CACHE = []


def get_nc():
    if not _NC_CACHE:
        _NC_CACHE.append(build_nc())
    return _NC_CACHE[0]


_HOST_CTX = {}


def _l2n(x, axis):
    n = np.sqrt((x * x).sum(axis, keepdims=True))
    return x / np.maximum(n, 1e-12)


def make_in_maps(batch, labels):
    import ml_dtypes
    X = np.asarray(batch, np.float32).reshape(B, C, S)
    xn = _l2n(X, 1)                       # [B, C, S]
    xm = _l2n(X.mean(2), 1)               # [B, C]
    sim2 = (xm @ xm.T).astype(np.float32)
    # AU[i, j, s] = relu(xm_i . xn_j[:, s]) : u for (i,j), v for (j,i)
    AU = np.maximum(np.einsum("ic,jcs->ijs", xm, xn,
                              optimize=True), 0.0).astype(np.float32)
    AUsum = AU.sum(2)                     # [i, j]
    sv = AUsum / (AUsum + 1e-5)           # sum of normalized v for pair (j,i)

    _HOST_CTX.clear()
    _HOST_CTX.update(labels=np.asarray(labels), sim2=sim2, sv=sv)

    ones2 = np.zeros((PP, 2), np.float32)
    ones2[0:S, 0] = 1.0
    ones2[S:PP, 1] = 1.0
    ones2 = ones2.astype(ml_dtypes.bfloat16)

    in_maps = []
    for k in range(NCORES):
        perm = (np.arange(NJ) + k * IPC) % B    # rotated col -> global sample
        xnb = np.ascontiguousarray(
            xn[perm].transpose(1, 0, 2).reshape(C, COLS)).astype(
                ml_dtypes.bfloat16)
        ug = np.zeros((PP, NT, NW), np.float32)
        vp = np.zeros((TBLK, 128, S), np.float32)
        for t in range(NT):
            for h in range(2):
                il = 2 * t + h
                i = (k * IPC + il) % B
                for d in range(1, ND + 1):
                    j = (i + d) % B
                    jw = d - 1 + h
                    ug[h * S:(h + 1) * S, t, jw] = AU[i, j]
                    q = (t % 2) * 64 + h * ND + (d - 1)
                    vp[t // 2, q] = AU[j, i] / (AUsum[j, i] + 1e-5)
        in_maps.append({
            "xnb": xnb,
            "ones2": ones2,
            "ug": np.ascontiguousarray(ug),
            "vp": np.ascontiguousarray(vp.transpose(1, 0, 2)),
        })
    return in_maps


def combine(results):
    labels = _HOST_CTX["labels"]
    sim2 = _HOST_CTX["sim2"]
    sv = _HOST_CTX["sv"]
    sim = np.full((B, B), np.nan, np.float32)
    for k in range(NCORES):
        dot = np.asarray(results[k]["dot"], np.float32)   # [128, TBLK]
        for b in range(TBLK):
            for q in range(128):
                t = 2 * b + q // 64
                r = q % 64
                h = r // 32
                d = (r % 32) + 1
                i = (k * IPC + 2 * t + h) % B
                j = (i + d) % B
                sim[i, j] = 0.5 * (dot[q, b] + sim2[i, j] * sv[i, j])
    miss = np.isnan(sim)
    sim[miss] = sim.T[miss]
    np.fill_diagonal(sim, 0.0)

    eye = np.eye(B, dtype=bool)
    same = labels[:, None] == labels[None, :]
    pos = same & ~eye
    neg = ~same
    minp = np.min(np.where(pos, sim, np.inf), 1)
    maxn = np.max(np.where(neg, sim, -np.inf), 1)
    nsel = neg & (sim + MARGIN > minp[:, None])
    psel = pos & (sim - MARGIN < maxn[:, None])
    valid = pos.any(1) & neg.any(1) & psel.any(1) & nsel.any(1)
    ps = np.where(psel, np.exp(-POS_W * (sim - THRESH)), 0.0).sum(1)
    ns = np.where(nsel, np.exp(NEG_W * (sim - THRESH)), 0.0).sum(1)
    pa = np.log1p(ps) / POS_W + np.log1p(ns) / NEG_W
    nv = max(float(valid.sum()), 1.0)
    return np.float32(float(np.where(valid, pa, 0.0).sum()) / nv)


def kernel(batch, labels):
    from concourse.bass_utils import run_bass_kernel_spmd
    nc = get_nc()
    in_maps = make_in_maps(batch, labels)
    res = run_bass_kernel_spmd(nc, in_maps, list(range(NCORES))).results
    return combine(res)


# revision 27
# speedup vs baseline: 1.0844x; 1.0844x over previous
"""Trainium2 Bass kernel for nn_Criterion_8761733284571.

Pairwise Wasserstein-attention similarity (1-step Sinkhorn) + multisimilarity
loss over a 64-sample batch.

v3 design ("no-bounce"), consolidated:
  - Symmetry: converged transport satisfies sim(i,j) = sim(j,i); only pairs
    with circular distance d = (j-i) mod 64 in [1,32] are computed (validated
    rel-err 3.5e-3 vs reference).  8 anchors/core x 32 d's = 256 pairs/core.
  - K stays in Gram layout [98=(anchor-half, s), (j-window, m)]: row-sums are
    contiguous 49-grouped DVE reduces; column reductions (den_c, SB) are PE
    ones-matmuls over the partition dim.  No big pair-major transpose.
  - Sinkhorn scale invariance: r = u/den0 (the u-normalization cancels in
    sum_m SB[m]*v[m]/den_c[m]); no partition-broadcast of per-pair sums.
  - Host precomputes normalized xn (bf16, only the 40 used j-columns),
    attention marginals u (Gram window layout) and v (pair-major, divided by
    vsum+1e-5), and runs the 64x64 multisimilarity reduction on gathered sims.
  - W and P2=W*sim live interleaved in one tensor so den_c|SB come from one
    7-chunk ones-matmul pass and one fused 196B-descriptor scatter per half.
"""

import os as _os

import numpy as np
from contextlib import ExitStack

import concourse.bass as bass
import concourse.bacc as bacc
import concourse.mybir as mybir
import concourse.tile as tile

F32 = mybir.dt.float32
BF16 = mybir.dt.bfloat16
AF = mybir.ActivationFunctionType
ALU = mybir.AluOpType
AX = mybir.AxisListType

B = 64          # batch
C = 128         # channels
S = 49          # spatial (7*7)
NCORES = 8
IPC = B // NCORES       # anchors per core = 8
NT = IPC // 2           # anchor-pair tiles = 4
ND = 32                 # circular distances per anchor
NW = ND + 1             # j-window width per tile = 33
WCOL = NW * S           # 1617
NJ = 2 * NT + ND        # j-columns actually used per core = 40
COLS = NJ * S           # 1960
TBLK = 2                # pair-major blocks of 128 pairs
PP = 2 * S              # 98 partitions in Gram layout

POS_W = 2.0
NEG_W = 40.0
MARGIN = 0.1
THRESH = 0.5


def _bc(ap, pos, count):
    """Insert a stride-0 (broadcast) dim of size `count` at position `pos`."""
    new = ap.ap[:pos] + [[0, count]] + ap.ap[pos:]
    return bass.AP(tensor=ap.tensor, offset=ap.offset, ap=new)


def _t11(ap):
    """Append a trailing [1, 1] dim (keeps the DVE 2x datapath with a
    broadcast operand; see the v2 kernel's measured bc-mul timings)."""
    return bass.AP(tensor=ap.tensor, offset=ap.offset, ap=ap.ap + [[1, 1]])


def _body(ctx, tc, io):
    nc = tc.nc

    psm = ctx.enter_context(tc.tile_pool(name="psm", bufs=1))
    ppg = ctx.enter_context(tc.tile_pool(name="ppg", bufs=2, space="PSUM"))
    ppr = ctx.enter_context(tc.tile_pool(name="ppr", bufs=2, space="PSUM"))

    # ---- constants ----
    cm20 = psm.tile([128, 1], F32)
    nc.vector.memset(cm20[:], -20.0)
    ones2 = psm.tile([PP, 2], BF16)

    # ---- load inputs (xnb chunked so tile 0's window lands first) ----
    xnb = psm.tile([C, COLS], BF16, tag="XNB")
    XCH = [(0, 537), (537, 1073), (1073, 1666), (1666, COLS)]
    for i, (x0, x1) in enumerate(XCH):
        eng = nc.sync if i % 2 == 0 else nc.scalar
        eng.dma_start(xnb[:, x0:x1], io["xnb"][:, x0:x1])
    nc.sync.dma_start(ones2[:], io["ones2"][:])
    uG = psm.tile([PP, NT, NW], F32)
    nc.scalar.dma_start(uG[:], io["ug"][:])
    vP = psm.tile([128, TBLK, S], F32)
    nc.sync.dma_start(vP[:], io["vp"][:])

    # ---- big per-tile tensors (Gram layout) ----
    simS = psm.tile([PP, NT, WCOL], BF16, tag="SIMS")
    KG = psm.tile([PP, NT, WCOL], BF16, tag="KG")
    WP = psm.tile([PP, NT, NW, 2, S], BF16, tag="WP")  # per jw: [W | W*sim]
    den0 = psm.tile([PP, NT, NW], F32)
    di0 = psm.tile([PP, NT, NW], F32)
    r0 = psm.tile([PP, NT, NW], BF16)
    dsbs = psm.tile([2, NT, 2 * WCOL], F32)            # den_c | SB interleaved
    dsP = psm.tile([128, TBLK, 2, S], F32)             # pair-major den_c, SB
    rdc = psm.tile([128, TBLK, S], F32)
    ct = psm.tile([128, TBLK, S], F32)
    dotv = psm.tile([128, TBLK], F32)

    # Gram PSUM pieces (2 banks each) and ones-reduce pieces (4 banks each)
    GP = [(0, 1024), (1024, WCOL)]
    RP = [(0, 1024), (1024, 2048), (2048, 3072), (3072, 2 * WCOL)]

    for t in range(NT):
        a0 = 2 * t * S
        w0 = (2 * t + 1) * S
        # Gram: [98 anchor-cols, 33-j window]; per PSUM piece: matmuls at
        # bank-aligned 512-subslices, then one copy (sim) + one exp (K)
        for (g0, g1) in GP:
            pg = ppg.tile([PP, 1024], F32, tag="pg")
            for f0 in range(g0, g1, 512):
                f1 = min(f0 + 512, g1)
                nc.tensor.matmul(pg[:, f0 - g0:f1 - g0],
                                 lhsT=xnb[:, a0:a0 + PP],
                                 rhs=xnb[:, w0 + f0:w0 + f1],
                                 start=True, stop=True)
            nc.scalar.copy(simS[:, t, g0:g1], pg[:, 0:g1 - g0])
            nc.scalar.activation(KG[:, t, g0:g1], pg[:, 0:g1 - g0], AF.Exp,
                                 bias=cm20[0:PP], scale=20.0)
        # den0[s, jw] = sum_m K ; r0 = u_raw / den0 (usum cancels downstream)
        nc.vector.tensor_reduce(
            den0[:, t], KG[:, t].rearrange("p (j m) -> p j m", m=S),
            axis=AX.X, op=ALU.add)
        nc.vector.reciprocal(di0[:, t], den0[:, t])
        nc.vector.tensor_mul(r0[:, t], uG[:, t], di0[:, t])
        # W = K * r0 (r0 broadcast along m); P2 = W * sim (gpsimd, off path)
        nc.vector.tensor_mul(
            WP[:, t, :, 0],
            KG[:, t].rearrange("p (j m) -> p j m", m=S),
            _bc(r0[:, t], 2, S))
        p2eng = nc.gpsimd if t < NT - 1 else nc.vector
        p2eng.tensor_mul(WP[:, t, :, 1], WP[:, t, :, 0],
                         simS[:, t].rearrange("p (j m) -> p j m", m=S))
        # den_c | SB: PE ones-reduce over partitions; per 4-bank PSUM piece:
        # matmuls at bank-aligned 512-subslices, then one big copy
        wp_flat = WP[:, t].rearrange("p j c m -> p (j c m)")
        for pi, (g0, g1) in enumerate(RP):
            pd = ppr.tile([2, 1024], F32, tag="pr")
            for f0 in range(g0, g1, 512):
                f1 = min(f0 + 512, g1)
                nc.tensor.matmul(pd[:, f0 - g0:f1 - g0], lhsT=ones2[:],
                                 rhs=wp_flat[:, f0:f1],
                                 start=True, stop=True)
            if pi % 2 == 0:
                nc.scalar.copy(dsbs[:, t, g0:g1], pd[:, 0:g1 - g0])
            else:
                nc.vector.tensor_copy(dsbs[:, t, g0:g1], pd[:, 0:g1 - g0])
        # scatter den_c/SB to pair-major [128, 2, 49]; one DMA per half
        b = t // 2
        qb = (t % 2) * 64
        for h in range(2):
            src = dsbs[h:h + 1, t].rearrange("p (j w) -> p j w", w=2 * S)
            src = bass.AP(tensor=src.tensor,
                          offset=src.offset + h * 2 * S,
                          ap=[src.ap[0], [src.ap[1][0], ND], src.ap[2]])
            nc.sync.dma_start(dsP[qb + h * ND:qb + (h + 1) * ND, b], src)

    # ---- pair-major finish: c = vP / den_c ; dot = sum_m SB * c ----
    for b in range(TBLK):
        nc.vector.reciprocal(rdc[:, b], dsP[:, b, 0])
        nc.vector.tensor_mul(ct[:, b], vP[:, b], rdc[:, b])
        nc.vector.tensor_mul(ct[:, b], ct[:, b], dsP[:, b, 1])
        nc.vector.tensor_reduce(dotv[:, b:b + 1], ct[:, b],
                                axis=AX.X, op=ALU.add)
        nc.scalar.dma_start(io["dot"][:, b:b + 1], dotv[:, b:b + 1])


def build_nc():
    nc = bacc.Bacc("TRN2", target_bir_lowering=False, debug=False)
    io = {}
    io["xnb"] = nc.declare_dram_parameter("xnb", [C, COLS], BF16, isOutput=False)
    io["ones2"] = nc.declare_dram_parameter("ones2", [PP, 2], BF16, isOutput=False)
    io["ug"] = nc.declare_dram_parameter("ug", [PP, NT, NW], F32, isOutput=False)
    io["vp"] = nc.declare_dram_parameter("vp", [128, TBLK, S], F32, isOutput=False)
    io["dot"] = nc.declare_dram_parameter("dot", [128, TBLK], F32, isOutput=True)
    with tile.TileContext(nc) as tc, ExitStack() as ctx:
        _body(ctx, tc, io)
    nc.compile()
    return nc


_NC_CACHE = []


def get_nc():
    if not _NC_CACHE:
        _NC_CACHE.append(build_nc())
    return _NC_CACHE[0]


_HOST_CTX = {}


def _l2n(x, axis):
    n = np.sqrt((x * x).sum(axis, keepdims=True))
    return x / np.maximum(n, 1e-12)


def make_in_maps(batch, labels):
    import ml_dtypes
    X = np.asarray(batch, np.float32).reshape(B, C, S)
    xn = _l2n(X, 1)                       # [B, C, S]
    xm = _l2n(X.mean(2), 1)               # [B, C]
    sim2 = (xm @ xm.T).astype(np.float32)
    # AU[i, j, s] = relu(xm_i . xn_j[:, s]) : u for (i,j), v for (j,i)
    AU = np.maximum(np.einsum("ic,jcs->ijs", xm, xn,
                              optimize=True), 0.0).astype(np.float32)
    AUsum = AU.sum(2)                     # [i, j]
    sv = AUsum / (AUsum + 1e-5)           # sum of normalized v for pair (j,i)

    _HOST_CTX.clear()
    _HOST_CTX.update(labels=np.asarray(labels), sim2=sim2, sv=sv)

    ones2 = np.zeros((PP, 2), np.float32)
    ones2[0:S, 0] = 1.0
    ones2[S:PP, 1] = 1.0
    ones2 = ones2.astype(ml_dtypes.bfloat16)

    in_maps = []
    for k in range(NCORES):
        perm = (np.arange(NJ) + k * IPC) % B    # rotated col -> global sample
        xnb = np.ascontiguousarray(
            xn[perm].transpose(1, 0, 2).reshape(C, COLS)).astype(
                ml_dtypes.bfloat16)
        ug = np.zeros((PP, NT, NW), np.float32)
        vp = np.zeros((TBLK, 128, S), np.float32)
        for t in range(NT):
            for h in range(2):
                il = 2 * t + h
                i = (k * IPC + il) % B
                for d in range(1, ND + 1):
                    j = (i + d) % B
                    jw = d - 1 + h
                    ug[h * S:(h + 1) * S, t, jw] = AU[i, j]
                    q = (t % 2) * 64 + h * ND + (d - 1)
                    vp[t // 2, q] = AU[j, i] / (AUsum[j, i] + 1e-5)
        in_maps.append({
            "xnb": xnb,
            "ones2": ones2,
            "ug": np.ascontiguousarray(ug),
            "vp": np.ascontiguousarray(vp.transpose(1, 0, 2)),
        })
    return in_maps


def combine(results):
    labels = _HOST_CTX["labels"]
    sim2 = _HOST_CTX["sim2"]
    sv = _HOST_CTX["sv"]
    sim = np.full((B, B), np.nan, np.float32)
    for k in range(NCORES):
        dot = np.asarray(results[k]["dot"], np.float32)   # [128, TBLK]
        for b in range(TBLK):
            for q in range(128):
                t = 2 * b + q // 64
                r = q % 64
                h = r // 32
                d = (r % 32) + 1
                i = (k * IPC + 2 * t + h) % B
                j = (i + d) % B
                sim[i, j] = 0.5 * (dot[q, b] + sim2[i, j] * sv[i, j])
    miss = np.isnan(sim)
    sim[miss] = sim.T[miss]
    np.fill_diagonal(sim, 0.0)

    eye = np.eye(B, dtype=bool)
    same = labels[:, None] == labels[None, :]
    pos = same & ~eye
    neg = ~same
    minp = np.min(np.where(pos, sim, np.inf), 1)
    maxn = np.max(np.where(neg, sim, -np.inf), 1)
    nsel = neg & (sim + MARGIN > minp[:, None])
    psel = pos & (sim - MARGIN < maxn[:, None])
    valid = pos.any(1) & neg.any(1) & psel.any(1) & nsel.any(1)
    ps = np.where(psel, np.exp(-POS_W * (sim - THRESH)), 0.0).sum(1)
    ns = np.where(nsel, np.exp(NEG_W * (sim - THRESH)), 0.0).sum(1)
    pa = np.log1p(ps) / POS_W + np.log1p(ns) / NEG_W
    nv = max(float(valid.sum()), 1.0)
    return np.float32(float(np.where(valid, pa, 0.0).sum()) / nv)


def kernel(batch, labels):
    from concourse.bass_utils import run_bass_kernel_spmd
    nc = get_nc()
    in_maps = make_in_maps(batch, labels)
    res = run_bass_kernel_spmd(nc, in_maps, list(range(NCORES))).results
    return combine(res)
